# revision 14
# baseline (speedup 1.0000x reference)
"""GCN encoder (nn_GCNEncoder) Trainium2 Bass kernel.

Math: with a fully-connected graph + self loops, gcn_norm gives the uniform
adjacency A = 1/N. Then A @ X broadcasts mean_n(X) to every node, so after
layer 1 the node features are constant within each graph and the whole GCN
collapses to a per-graph vector chain:

  locbar[b] = mean_n locs[b, n, :]                       (R^2)
  g0[b]     = locbar[b] @ W_init + b_init                (R^D)
  g1        = relu(g0 @ Ws[0] + bs[0]);  g2 = relu(g1 @ Ws[1] + bs[1])
  g3        = g2 @ Ws[2] + bs[2]
  init_h[b, n, :]  = locs[b, n, :] @ W_init + b_init
  h_final[b, n, :] = init_h[b, n, :] + g3[b, :]

The kernel is OUTPUT-STORE bound. Both outputs are quantized on-device to
uint8 (bias 128): the tolerance is 2e-2 and the quant step is ~4e-3 of
scale, so stores shrink 4x vs f32 (6.55 MB/core). Casts truncate toward
zero, so quantization computes trunc(x*127/S + 128.5) = round-half-up.
A SINGLE scale S = max(sup|h|, sup|init_h|) per 128-graph half serves both
outputs; sup|.| over the locs box [0,1]^2 is exact via its 4 corners
(corners input + device g3), so no uint8 overflow is possible. Scales ship
to the host via the tiny "scales" output for dequantization.

Device strategy (per core: 256 graphs = 25600 tokens, 8 chunks x 32 graphs):
 - ONE bf16 matmul per 128-token tile produces BOTH outputs (K=128 rows:
   locs hi/lo terms + ones + 3x32 sel/g3 rows; rows 10..31 zero pad so the
   sel block sits 32-aligned). lhsT is one persistent SBUF strip
   [128, 25600] (51.2 KB/partition): master rows loaded once, chunk-
   invariant sel block loaded once and replicated 8x by on-chip copies.
 - rhs is one [128, 8*256] slab: static W/bias rows loaded once; the 96 g3
   rows per chunk are written by small SWDGE DMAs right after the g3 chain
   computes each half - the main loop itself issues NO load DMAs.
 - PSUM: [128,1024] 2-bank quad tiles (4 tiles x [init|final]); ONE engine
   op per quad (alternating DVE tensor_scalar / ACT activation, 23/27
   split) quantizes+evacuates all 8 output blocks, amortizing the ~180 ns
   PSUM access latency.
 - Stores: both outputs ride ONE fused DMA per 20-tile group (655 KB) in a
   partition-major DRAM layout (s p o u d) giving 2560 B contiguous runs
   (>=512 B keeps SDMA at line rate); rings alternate sync/scalar. Host
   gather undoes the permutation and dequantizes.
 - Measured on trn2 (8 cores): 47 us steady-state main loop (vs 80 us for
   the f32-store baseline); HW rel err 1.0e-2 (quant-dominated, gate 2e-2).
"""

import numpy as np
from contextlib import ExitStack

import concourse.bass as bass
import concourse.mybir as mybir
import concourse.tile as tile
from concourse.bass_utils import run_bass_kernel_spmd

F32 = mybir.dt.float32
BF16 = mybir.dt.bfloat16
U8 = mybir.dt.uint8
AF = mybir.ActivationFunctionType

B, N, D, L = 2048, 100, 128, 3
NCORES = 8
BG = B // NCORES          # 256 graphs per core
T = BG * N                # 25600 tokens per core
NT = T // 128             # 200 token tiles per core
CH = 8                    # chunks per core
TPC = NT // CH            # 25 tiles per chunk
GPC = BG // CH            # 32 graphs per chunk
KB = 10                   # base lhsT rows (locs hi/lo + ones)
KS = 32                   # sel/g3 rows start here (32-aligned engine APs)
KK = KS + 3 * GPC         # 128 contraction rows; 10..31 zero on both sides
SG = 20                   # tiles per store group (2560 tokens, 1.25 MB)
NSG = NT // SG            # 25 store groups


def _split_multiwaits(nc, max_waits=1):
    """The walrus build in this container rejects instructions carrying more
    than one sync-wait command. Split extras into single-wait NoOps inserted
    immediately before the instruction (same engine, so sequencer order
    preserves semantics exactly)."""
    cnt = 0
    for f in nc.m.functions:
        for b in f.blocks:
            il = b.instructions
            i = 0
            while i < len(il):
                ins = il[i]
                si = ins.sync_info
                if si is not None and si.on_wait and len(si.on_wait) > max_waits:
                    waits = list(si.on_wait)
                    for w in waits[:-max_waits]:
                        nop = mybir.InstNoOp(name=f"I-SWAIT-{cnt}", ins=[], outs=[])
                        cnt += 1
                        nop.engine = ins.engine
                        nop.sync_info = mybir.SyncInfo(on_wait=[w], on_update=[])
                        il.insert(i, nop)
                        i += 1
                    ins.sync_info = mybir.SyncInfo(
                        on_wait=waits[-max_waits:],
                        on_update=list(si.on_update or []))
                i += 1
    return cnt


def _build_program(split=True, reps=1, timing=False):
    # timing=True: big outputs become Internal DRAM scratch so repeated-
    # execution wall-clock timing doesn't pay the output download; the
    # stores still run identically.
    nc = bass.Bass("TRN2", target_bir_lowering=False, debug=False,
                   num_devices=NCORES)

    ins = {}
    for name, shape, dt in [
        ("master", [KS, T], BF16),
        ("selconst", [3 * GPC, 128 * TPC], BF16),
        ("rhs_init8", [KK, CH * 256], BF16),
        ("locs_gm", [BG, 2 * N], F32),
        ("wmean", [2, D], F32),
        ("bcol", [D, 1], F32),
        ("bsT", [D, L], F32),
        ("Ws", [L, D, D], F32),
        ("ident", [D, D], F32),
        ("corners", [D, 4], F32),
        ("qconst", [D, 2], F32),
    ]:
        ins[name] = nc.dram_tensor(name, shape, dt, kind="ExternalInput").ap()

    # Outputs are quantized to uint8 biased by 128 (tolerance is 2e-2; the
    # quant step is ~4e-3 of scale). f32->int casts truncate toward zero, so
    # quantize as trunc(x*127/S + 128.5) = round-half-up (arg always > 0).
    # init_h's scale is host-known (exact sup of |x wx + y wy + b| over the
    # [0,1]^2 locs box via its corners); h's scale is computed on device per
    # 128-graph half from the same corners + g3, shipped back via "scales".
    # DRAM order (s, p, u, d): each partition's store run is SG*D = 2560
    # contiguous bytes (>=512B keeps SDMA at line rate).
    okind = "Internal" if timing else "ExternalOutput"
    out_both = nc.dram_tensor("out_both", [2, T, D], U8, kind=okind).ap()
    out_scales = nc.dram_tensor("scales", [1, 2], F32, kind="ExternalOutput").ap()
    # store-group view: [NSG, 128, 2, SG, D]; one DMA stores BOTH outputs
    outB_r = out_both.rearrange("o (s p u) d -> s p o u d", p=128, u=SG)

    with tile.TileContext(nc) as tc, ExitStack() as ctx:
        const = ctx.enter_context(tc.tile_pool(name="const", bufs=1))

        ident_sb = const.tile([D, D], F32, tag="ident")
        nc.sync.dma_start(ident_sb[:], ins["ident"][:])
        wmean_sb = const.tile([2, D], F32, tag="wmean")
        nc.sync.dma_start(wmean_sb[:], ins["wmean"][:])
        bcol_sb = const.tile([D, 1], F32, tag="bcol")
        nc.sync.dma_start(bcol_sb[:], ins["bcol"][:])
        bsT_sb = const.tile([D, L], F32, tag="bsT")
        nc.sync.dma_start(bsT_sb[:], ins["bsT"][:])
        ws_sb = []
        for l in range(L):
            w = const.tile([D, D], F32, tag=f"ws{l}")
            nc.sync.dma_start(w[:], ins["Ws"][l])
            ws_sb.append(w)
        corn_sb = const.tile([D, 4], F32, tag="corners")
        nc.sync.dma_start(corn_sb[:], ins["corners"][:])
        qc_sb = const.tile([D, 2], F32, tag="qconst")
        nc.sync.dma_start(qc_sb[:], ins["qconst"][:])
        ones_row = const.tile([1, D], F32, tag="ones_row")
        nc.vector.memset(ones_row[:], 1.0)

        # The ENTIRE lhsT strip lives in SBUF (51.2 KB/partition): master
        # rows once from HBM; the chunk-invariant sel block loaded once and
        # replicated 8x on-chip by doubling copies. The main loop then has
        # ZERO load DMAs - its only DMAs are the 10 fused group stores.
        bigt = const.tile([KK, 128 * NT], BF16, tag="bigt")
        nc.sync.dma_start(bigt[0:KS, :], ins["master"][:])
        nc.scalar.dma_start(bigt[KS:KK, 0:128 * TPC], ins["selconst"][:])
        wsel = 128 * TPC
        for c in range(1, CH):
            # replicate the chunk-invariant sel block to chunk c's columns
            # (one-time). Scalar HWDGE ring: keeps DVE free for evacuation
            # and the SWDGE queue free for the latency-critical g3 rows.
            nc.scalar.dma_start(bigt[KS:KK, wsel * c:wsel * (c + 1)],
                                bigt[KS:KK, 0:wsel])
        # one rhs slab [KK, CH*256]: static rows loaded once; the 96 g3 rows
        # per chunk are filled by ONE SWDGE DMA per 128-graph half
        slab = const.tile([KK, CH * 256], BF16, tag="slab")
        nc.scalar.dma_start(slab[:], ins["rhs_init8"][:])

        # ---------------- persistent tiles for the g3 chain ----------------
        gsb = ctx.enter_context(tc.tile_pool(name="gsb", bufs=1))
        locbarT = gsb.tile([2, BG], F32, tag="locbarT")
        g3gm = gsb.tile([128, BG], F32, tag="g3gm")
        # 3 bf16 terms of g3 side by side in one tile: term t in cols
        # [256t, 256t+256) -> one combined per-chunk rhs DMA (SWDGE emission
        # costs ~1us per dma_start regardless of size)
        g3b3 = gsb.tile([128, 3 * BG], BF16, tag="g3b3")
        inv_h = gsb.tile([128, 2], F32, tag="inv_h")   # 127/S_h per half
        sc_sb = gsb.tile([1, 2], F32, tag="sc_sb")     # S_h per half (to host)

        # One [128,1024] PSUM tile = 2 banks (init cols 0:512, final cols
        # 512:1024) per 4-tile quad; a single engine op evacuates BOTH
        # outputs (they share one quant scale), alternating DVE/ACT per
        # quad. 3 bufs x 2 banks + 2 banks for the g3 chain's gps pool = 8.
        psIFpool = ctx.enter_context(
            tc.tile_pool(name="psIF", bufs=3, space="PSUM"))
        sIFpool = ctx.enter_context(tc.tile_pool(name="sIF", bufs=4))

        def g3_chain():
            # per-graph g3 chain (fp32). gps uses a single rotating tag so it
            # costs 2 PSUM banks (each tag x buf = one bank); main pool has 6.
            with tc.tile_pool(name="gps", bufs=2, space="PSUM") as gps, \
                 tc.tile_pool(name="gtmp", bufs=2) as gtmp:
                # Whole chain per 128-graph half so chunk 0 (graphs 0..31)
                # unblocks early; half 1 computes under the main loop.
                for h in range(2):
                    hs = slice(128 * h, 128 * (h + 1))
                    lg = gtmp.tile([128, 2 * N], F32, tag="lg")
                    nc.sync.dma_start(lg[:], ins["locs_gm"][hs, :])
                    lb = gtmp.tile([128, 2], F32, tag="lb")
                    lgk = lg[:].rearrange("p (n k) -> p k n", k=2)
                    for k in range(2):
                        nc.vector.tensor_reduce(
                            lb[:, k:k + 1], lgk[:, k:k + 1, :],
                            axis=mybir.AxisListType.X, op=mybir.AluOpType.add)
                    tpb = gps.tile([128, 128], F32, tag="gp")
                    tp = tpb[0:2, :]
                    nc.tensor.transpose(tp, lb[:], ident_sb[:])
                    nc.vector.tensor_copy(locbarT[:, hs], tp)

                    mp = gps.tile([128, 128], F32, tag="gp")
                    nc.tensor.matmul(mp[:], wmean_sb[:], locbarT[:, hs],
                                     start=True, stop=True)
                    g_prev = gsb.tile([128, 128], F32, tag=f"g0h{h}")
                    nc.scalar.activation(g_prev[:], mp[:], AF.Identity,
                                         bias=bcol_sb[:, 0:1])
                    for l in range(L):
                        pp = gps.tile([128, 128], F32, tag="gp")
                        nc.tensor.matmul(pp[:], ws_sb[l][:], g_prev[:],
                                         start=True, stop=True)
                        g_next = gsb.tile([128, 128], F32, tag=f"g{l + 1}h{h}")
                        nc.scalar.activation(
                            g_next[:], pp[:],
                            AF.Relu if l < L - 1 else AF.Identity,
                            bias=bsT_sb[:, l:l + 1])
                        g_prev = g_next
                    tq = gps.tile([128, 128], F32, tag="gp")
                    nc.tensor.transpose(tq[:], g_prev[:], ident_sb[:])
                    nc.vector.tensor_copy(g3gm[:, hs], tq[:])

                    # 3-term bf16 split of g3 (residual after 3 terms ~2^-26)
                    rcur_ap = g3gm[:, hs]
                    for t in range(3):
                        dst = g3b3[:, 256 * t + 128 * h:256 * t + 128 * h + 128]
                        nc.vector.tensor_copy(dst, rcur_ap)
                        if t < 2:
                            up = gtmp.tile([128, 128], F32, tag="up")
                            nc.vector.tensor_copy(up[:], dst)
                            rnext = gtmp.tile([128, 128], F32, tag=f"r{t}")
                            nc.vector.tensor_tensor(rnext[:], rcur_ap, up[:],
                                                    op=mybir.AluOpType.subtract)
                            rcur_ap = rnext[:]

                    # g3 rows for this half's 4 chunks -> rhs slab. Simple
                    # 2D APs only: multi-dim SWDGE APs miss dependency edges
                    # in this Tile build (race). Chunk-major so chunk 4h's
                    # rows land first.
                    for c in range(4 * h, 4 * h + 4):
                        pb = GPC * (c % 4)
                        for t in range(3):
                            nc.gpsimd.dma_start(
                                slab[KS + GPC * t:KS + GPC * (t + 1),
                                     256 * c + 128:256 * c + 256],
                                g3b3[pb:pb + GPC,
                                     256 * t + 128 * h:256 * t + 128 * h + 128])

                    # S_h[half] = max_{g,d,c} |corner[d,c] + g3[d,g]| (exact
                    # sup of |h| over the locs box -> no u8 overflow)
                    mc = []
                    for cr in range(4):
                        t4 = gtmp.tile([128, 128], F32, tag="qt")
                        # |g3 + corner| on ACT (walrus has no abs_max reduce)
                        nc.scalar.activation(t4[:], g3gm[:, hs], AF.Abs,
                                             bias=corn_sb[:, cr:cr + 1])
                        mr = gtmp.tile([128, 1], F32, tag=f"qm{cr}")
                        nc.vector.tensor_reduce(
                            mr[:], t4[:], axis=mybir.AxisListType.X,
                            op=mybir.AluOpType.max)
                        mc.append(mr)
                    m01 = gtmp.tile([128, 1], F32, tag="qm01")
                    nc.vector.tensor_tensor(m01[:], mc[0][:], mc[1][:],
                                            op=mybir.AluOpType.max)
                    m23 = gtmp.tile([128, 1], F32, tag="qm23")
                    nc.vector.tensor_tensor(m23[:], mc[2][:], mc[3][:],
                                            op=mybir.AluOpType.max)
                    mall = gtmp.tile([128, 1], F32, tag="qmall")
                    nc.vector.tensor_tensor(mall[:], m01[:], m23[:],
                                            op=mybir.AluOpType.max)
                    # fold in the init_h sup so ONE scale covers both
                    # outputs (they are within ~2% of each other anyway)
                    nc.vector.tensor_scalar_max(mall[:], mall[:],
                                                qc_sb[:, 1:2])
                    # cross-partition max via PE transpose, then 127/S and
                    # partition-broadcast via a K=1 matmul against ones
                    tsp = gps.tile([128, 128], F32, tag="gp")
                    nc.tensor.transpose(tsp[0:1, :], mall[:], ident_sb[:])
                    s1 = gtmp.tile([1, 1], F32, tag="qs1")
                    nc.vector.tensor_reduce(
                        s1[:], tsp[0:1, :], axis=mybir.AxisListType.X,
                        op=mybir.AluOpType.max)
                    nc.vector.tensor_copy(sc_sb[0:1, h:h + 1], s1[:])
                    i0 = gtmp.tile([1, 1], F32, tag="qi0")
                    nc.vector.tensor_scalar_mul(i0[:], s1[:], 1.0 / 127.0)
                    i1 = gtmp.tile([1, 1], F32, tag="qi1")
                    nc.vector.reciprocal(i1[:], i0[:])
                    bb = gps.tile([128, 128], F32, tag="gp")
                    nc.tensor.matmul(bb[:, 0:1], ones_row[:], i1[:],
                                     start=True, stop=True)
                    nc.vector.tensor_copy(inv_h[:, h:h + 1], bb[:, 0:1])

        def rep_body():
            # full per-invocation body (g3 chain + main loop) so reps-timing
            # measures the whole invocation, not just the main loop
            g3_chain()
            nc.scalar.dma_start(out_scales[:], sc_sb[:])
            main_body(nc, tc, ins, bigt, slab, psIFpool,
                      sIFpool, outB_r, inv_h)

        if reps > 1:
            # timing builds: loop only the main body (the g3 chain inside
            # For_i trips walrus codegen); measures steady-state main loop
            g3_chain()
            nc.scalar.dma_start(out_scales[:], sc_sb[:])
            with tc.For_i(0, reps, 1):
                main_body(nc, tc, ins, bigt, slab, psIFpool,
                          sIFpool, outB_r, inv_h)
        else:
            rep_body()

    if split:
        _split_multiwaits(nc)
    return nc


def main_body(nc, tc, ins, bigt, slab, psIFpool,
              sIFpool, outB_r, inv_h):
        psIF = sIF = None
        for c in range(CH):
            rh = slab[:, 256 * c:256 * (c + 1)]
            for i in range(TPC):
                ti = TPC * c + i
                q = ti % 4
                if q == 0:
                    psIF = psIFpool.tile([128, 1024], F32, tag="psIF")
                # ONE N=256 matmul per tile writes [init|final] into its
                # 256-col window of the 2-bank quad tile; ONE engine op per
                # quad evacuates all 4 tiles' both outputs (single scale)
                lhs = bigt[:, 128 * ti:128 * (ti + 1)]
                nc.tensor.matmul(psIF[:, 256 * q:256 * (q + 1)],
                                 lhs, rh, start=True, stop=True)
                if q == 3:
                    grp = ti // 4
                    sgrp = grp % (SG // 4)
                    if sgrp == 0:
                        sIF = sIFpool.tile([128, 2 * SG * 128], U8, tag="sIF")
                    half = ti // (NT // 2)
                    dst = sIF[:].rearrange("p (o u d) -> p o u d",
                                           o=2, u=SG)[
                        :, :, 4 * sgrp:4 * sgrp + 4, :]
                    srcv = psIF[:].rearrange("p (k o d) -> p o k d",
                                             k=4, o=2)
                    # DVE's 2-op tensor_scalar is ~15% slower than ACT's
                    # activation here -> give DVE 23 of the 50 quads
                    if (grp * 23) % 50 < 23:
                        nc.vector.tensor_scalar(
                            dst, srcv, scalar1=inv_h[:, half:half + 1],
                            scalar2=128.5,
                            op0=mybir.AluOpType.mult, op1=mybir.AluOpType.add)
                    else:
                        nc.scalar.activation(
                            dst, srcv, AF.Copy,
                            bias=128.5, scale=inv_h[:, half:half + 1])
                    if sgrp == SG // 4 - 1:
                        sg = grp // (SG // 4)
                        sIF_r = sIF[:].rearrange("p (o u d) -> p o u d",
                                                 o=2, u=SG)
                        eng = nc.sync if sg % 2 == 0 else nc.scalar
                        eng.dma_start(outB_r[sg], sIF_r)


def _bf_split(x, n=2):
    import ml_dtypes
    outs = []
    r = np.asarray(x, dtype=np.float32)
    for _ in range(n):
        h = r.astype(ml_dtypes.bfloat16)
        outs.append(h)
        r = r - h.astype(np.float32)
    return outs


def _prep_core_inputs(locs, W_init, b_init, Ws, bs):
    """Host-side shard + constant prep. Returns list of per-core input maps."""
    import ml_dtypes
    bfdt = ml_dtypes.bfloat16
    locs = np.ascontiguousarray(locs, dtype=np.float32)
    W_init = np.asarray(W_init, dtype=np.float32)
    b_init = np.asarray(b_init, dtype=np.float32)
    Ws = np.ascontiguousarray(Ws, dtype=np.float32)
    bs = np.asarray(bs, dtype=np.float32)

    # selconst[j, u] = 1 iff chunk-local token u belongs to chunk-graph j
    u = np.arange(128 * TPC)
    sel = (u[None, :] // N == np.arange(GPC)[:, None]).astype(bfdt)
    selconst = np.ascontiguousarray(np.concatenate([sel, sel, sel], axis=0))

    Wh, Wl = _bf_split(W_init)
    bh, bl = _bf_split(b_init)
    rhs_rows = [Wh[0], Wh[1], Wl[0], Wl[1], Wh[0], Wh[1], Wl[0], Wl[1], bh, bl]
    rhs_init = np.zeros((KK, 256), dtype=bfdt)
    for r, row in enumerate(rhs_rows):
        rhs_init[r, 0:128] = row
        rhs_init[r, 128:256] = row
    rhs_init8 = np.ascontiguousarray(np.tile(rhs_init, (1, CH)))

    wmean = np.ascontiguousarray(W_init / np.float32(N))
    bcol = np.ascontiguousarray(b_init.reshape(D, 1))
    bsT = np.ascontiguousarray(bs.T)
    ident = np.eye(D, dtype=np.float32)

    # quantization constants: corners of init_h over the [0,1]^2 locs box
    wx, wy = W_init[0], W_init[1]
    corners = np.stack([b_init, b_init + wx, b_init + wy, b_init + wx + wy],
                       axis=1).astype(np.float32)            # [D, 4]
    s_init = float(max(np.abs(corners).max(), 1e-30))
    qconst = np.zeros((D, 2), dtype=np.float32)
    qconst[:, 0] = np.float32(127.0) / np.float32(s_init)
    qconst[:, 1] = np.float32(s_init)

    in_maps = []
    for k in range(NCORES):
        lc = locs[BG * k:BG * (k + 1)]          # [256, 100, 2]
        lx, ly = lc[:, :, 0].ravel(), lc[:, :, 1].ravel()
        lxh, lxl = _bf_split(lx)
        lyh, lyl = _bf_split(ly)
        ones = np.ones(T, dtype=bfdt)
        master = np.zeros((KS, T), dtype=np.float32)
        for r, row in enumerate([lxh, lyh, lxh, lyh, lxl, lyl, lxl, lyl,
                                 ones, ones]):
            master[r] = row
        in_maps.append({
            "master": np.ascontiguousarray(master.astype(bfdt)),
            "selconst": selconst,
            "rhs_init8": rhs_init8,
            "locs_gm": np.ascontiguousarray(lc.reshape(BG, 2 * N)),
            "wmean": wmean,
            "bcol": bcol,
            "bsT": bsT,
            "Ws": Ws,
            "ident": ident,
            "corners": np.ascontiguousarray(corners),
            "qconst": qconst,
        })
    return in_maps, s_init


_CACHED_NC = None


def _get_nc():
    global _CACHED_NC
    if _CACHED_NC is None:
        _CACHED_NC = _build_program()
    return _CACHED_NC


def _decode_out(arr, s0, s1):
    """u8 [T, D] in (s p u d) permuted order -> f32 [BG, N, D] token order,
    dequantized with per-half scales s0 (tokens < T/2) and s1."""
    a = np.asarray(arr).reshape(NSG, 128, SG, D)
    a = a.transpose(0, 2, 1, 3).astype(np.float32)
    a -= np.float32(128.0)
    a = a.reshape(2, T // 2 // N, N, D)
    a[0] *= np.float32(s0 / 127.0)
    a[1] *= np.float32(s1 / 127.0)
    return a.reshape(BG, N, D)


def kernel(locs, W_init, b_init, Ws, bs, _trace=False):
    nc = _get_nc()
    in_maps, s_init = _prep_core_inputs(locs, W_init, b_init, Ws, bs)
    res = run_bass_kernel_spmd(nc, in_maps, list(range(NCORES)), trace=_trace)
    hs, ihs = [], []
    for k in range(NCORES):
        sc = np.asarray(res.results[k]["scales"], np.float32).ravel()
        ob = np.asarray(res.results[k]["out_both"])
        ihs.append(_decode_out(ob[0], sc[0], sc[1]))
        hs.append(_decode_out(ob[1], sc[0], sc[1]))
    h = np.concatenate(hs, axis=0)
    init_h = np.concatenate(ihs, axis=0)
    if _trace:
        return (h, init_h), res
    return (h, init_h)

